# revision 7
# baseline (speedup 1.0000x reference)
"""Trainium2 Bass kernel for 5-relation GAT (nn_GAT_76716705841462). v2

Strategy: destination-sharded, collective-free, bf16 data path.
  * Host prep (sharding only): transpose x, sort each relation's REAL edges
    by destination, bucket into 128-dst windows, pad each (window, relation)
    bucket to (B1+B2)*128 edge slots.  Self-loops are NOT placed in the edge
    stream; they are handled by a per-window diagonal block fed from a
    sequentially-loaded table (no gather descriptors).  dma_gather indices
    are int16 (<32768), so edge slots split into B1 "lo" blocks gathered
    from T[0:LOW_CAP] and B2 "hi" blocks gathered from T[H0:], with srcs in
    the overlap band assigned to whichever side has room.
  * Device, phase A (replicated on every core): bf16 node table
    T[n] = [h0(128) | 1 | h1(129..257) | 1 | a_src(2) | a_dst(2) | pad]
    (384 bf16 = 768B rows) via xT.T @ [Wsrc_h | 0 | Vs | Vd | 0] where
    Vs = per-head W_src @ att_src, Vd = W_dst @ att_dst are built on device.
    h_dst is never materialized; segment-max subtraction is skipped (logits
    bounded ~10, exp is safe, softmax unchanged).  Also builds, in
    core-local window order, T_self (same rows, read sequentially for the
    self-loop diagonal block) and Twin_a (256B bf16 rows holding a_dst for
    the per-slot a_dst gather).
  * Device, phase B, per window: 3 merged dma_gathers (lo, hi, a_dst; all 5
    relations per call); asum = a_src + a_dst on DVE; leaky-relu and exp on
    the Activation engine; weighted one-hot W_h[e,n] =
    (iota[n]==dst[e]) * expl[e,h] in one fused bf16 tensor_scalar (DVE 4x
    mode); bf16 TensorE matmul W_h.T @ [G_h | 1] accumulates numerator and
    softmax denominator in PSUM over the self block + B1+B2 edge blocks.
    Per-relation division runs on DVE (reciprocal) + Activation (scaled
    copy); the +1e-16 eps is dropped (self-loop keeps denominators > 0).
"""

import os

import numpy as np
import ml_dtypes

import concourse.bacc as bacc
import concourse.bass as bass
import concourse.mybir as mybir
import concourse.tile as tile
from concourse.library_config import mlp

P = 128
H = 2
C = 128
D = 256
R = 5
TW = 384          # bf16 T row: [h0|1|h1|1|a_src(2)|a_dst(2)|pad] = 768B
A_OFF = 258       # a_src at 258:260, a_dst at 260:262
AW = 128          # bf16 Twin_a row: [a_dst(2)|pad] = 256B
NEG = 0.2

f32 = mybir.dt.float32
bf16 = mybir.dt.bfloat16
i16 = mybir.dt.int16
BF = ml_dtypes.bfloat16

_CACHE = {}
_RUN_KWARGS = {}      # test harness may set e.g. {"trace": True}
_LAST_RESULT = None   # BassKernelResults of the last run (for profiling)


def build_program(n_tiles, t_rows, w_pc, B1, B2, low_cap, h0, nw_p,
                  num_devices):
    BT = B1 + B2
    nc = bacc.Bacc("TRN2", target_bir_lowering=False, debug=False,
                   num_devices=num_devices)

    xT = nc.dram_tensor("xT", [D, nw_p], bf16, kind="ExternalInput")
    xT_local = nc.dram_tensor("xT_local", [D, w_pc * P], bf16,
                              kind="ExternalInput")
    Wsrc = nc.dram_tensor("Wsrc", [D, D], f32, kind="ExternalInput")
    Wdst = nc.dram_tensor("Wdst", [D, D], f32, kind="ExternalInput")
    atts = nc.dram_tensor("atts", [1, D], f32, kind="ExternalInput")
    attd = nc.dram_tensor("attd", [1, D], f32, kind="ExternalInput")
    bias_in = nc.dram_tensor("bias_in", [1, D], f32, kind="ExternalInput")
    iota_in = nc.dram_tensor("iota_in", [P, P], bf16, kind="ExternalInput")
    piota_in = nc.dram_tensor("piota_in", [P, 1], f32, kind="ExternalInput")
    lo_cols = R * B1 * P // 16
    hi_cols = R * B2 * P // 16
    ad_cols = R * BT * P // 16
    lo16 = nc.dram_tensor("lo16", [w_pc * P, lo_cols], i16,
                          kind="ExternalInput")
    hi16 = nc.dram_tensor("hi16", [w_pc * P, hi_cols], i16,
                          kind="ExternalInput")
    ad16 = nc.dram_tensor("ad16", [w_pc * P, ad_cols], i16,
                          kind="ExternalInput")
    drlx = nc.dram_tensor("drlx", [w_pc * P, R * BT], f32,
                          kind="ExternalInput")
    y = nc.dram_tensor("y", [w_pc * P, D], f32, kind="ExternalOutput")

    T = nc.dram_tensor("T", [t_rows, TW], bf16)
    Tself = nc.dram_tensor("Tself", [w_pc * P, TW], bf16)
    Twin_a = nc.dram_tensor("Twin_a", [w_pc * P, AW], bf16)

    # ---- TileContext 1: setup + table build (exit = all-engine barrier) ----
    with tile.TileContext(nc) as tc:
        with (
            tc.tile_pool(name="setup", bufs=1) as su,
            tc.tile_pool(name="ps_su", bufs=1, space="PSUM") as psu,
        ):
            ws_h = [su.tile([P, D], f32, name=f"ws_h{k}") for k in range(2)]
            wd_h = [su.tile([P, D], f32, name=f"wd_h{k}") for k in range(2)]
            for k in range(2):
                nc.sync.dma_start(ws_h[k][:], Wsrc[k * P:(k + 1) * P, :])
                nc.sync.dma_start(wd_h[k][:], Wdst[k * P:(k + 1) * P, :])
            ones1 = su.tile([1, P], f32)
            nc.vector.memset(ones1[:], 1.0)
            atts_sb = su.tile([1, D], f32)
            attd_sb = su.tile([1, D], f32)
            nc.sync.dma_start(atts_sb[:], atts[:])
            nc.sync.dma_start(attd_sb[:], attd[:])
            atts_bc = su.tile([P, D], f32)
            attd_bc = su.tile([P, D], f32)
            for row_sb, bc in ((atts_sb, atts_bc), (attd_sb, attd_bc)):
                ps_bc = psu.tile([P, D], f32, name="ps_bc", tag="ps_bc")
                nc.tensor.matmul(out=ps_bc[:], lhsT=ones1[:], rhs=row_sb[:],
                                 start=True, stop=True)
                nc.vector.tensor_copy(bc[:], ps_bc[:])

            rhs_k = [su.tile([P, TW], bf16, name=f"rhs_k{k}")
                     for k in range(2)]
            for k in range(2):
                rk = rhs_k[k]
                nc.vector.memset(rk[:], 0.0)
                nc.vector.tensor_copy(rk[:, 0:C], ws_h[k][:, 0:C])
                nc.vector.tensor_copy(rk[:, C + 1:2 * C + 1], ws_h[k][:, C:D])
                for h in range(H):
                    for src_w, src_bc, col in (
                        (ws_h[k], atts_bc, A_OFF + h),
                        (wd_h[k], attd_bc, A_OFF + 2 + h),
                    ):
                        scratch = su.tile([P, C], f32, name="vscr",
                                          tag="vscr", bufs=2)
                        nc.vector.tensor_tensor(
                            out=scratch[:],
                            in0=src_w[:, h * C:(h + 1) * C],
                            in1=src_bc[:, h * C:(h + 1) * C],
                            op=mybir.AluOpType.mult)
                        vcol = su.tile([P, 1], f32, name="vcol", tag="vcol",
                                       bufs=2)
                        nc.vector.tensor_reduce(
                            out=vcol[:], in_=scratch[:],
                            axis=mybir.AxisListType.X,
                            op=mybir.AluOpType.add)
                        nc.vector.tensor_copy(rk[:, col:col + 1], vcol[:])

            with (
                tc.tile_pool(name="sb_tbl", bufs=3) as stp,
                tc.tile_pool(name="ps_tbl", bufs=2, space="PSUM") as ptp,
            ):
                def build_rows(src_dram, t, dst_dram, want_wina, parity):
                    xk0 = stp.tile([P, P], bf16, name="xk0")
                    xk1 = stp.tile([P, P], bf16, name="xk1")
                    nc.sync.dma_start(xk0[:], src_dram[0:P, t * P:(t + 1) * P])
                    nc.sync.dma_start(xk1[:], src_dram[P:D, t * P:(t + 1) * P])
                    ps_t = ptp.tile([P, TW], f32, name="ps_t")
                    nc.tensor.matmul(out=ps_t[:], lhsT=xk0[:], rhs=rhs_k[0][:],
                                     start=True, stop=False)
                    nc.tensor.matmul(out=ps_t[:], lhsT=xk1[:], rhs=rhs_k[1][:],
                                     start=False, stop=True)
                    stg = stp.tile([P, TW], bf16, name="stg")
                    if parity:
                        nc.vector.tensor_copy(stg[:], ps_t[:])
                    else:
                        nc.scalar.copy(stg[:], ps_t[:])
                    nc.vector.memset(stg[:, C:C + 1], 1.0)
                    nc.vector.memset(stg[:, 2 * C + 1:2 * C + 2], 1.0)
                    nc.sync.dma_start(dst_dram[t * P:(t + 1) * P, :], stg[:])
                    if want_wina:
                        stga = stp.tile([P, AW], bf16, name="stga")
                        nc.vector.memset(stga[:], 0.0)
                        nc.vector.tensor_copy(stga[:, 0:2], stg[:, 260:262])
                        nc.sync.dma_start(Twin_a[t * P:(t + 1) * P, :],
                                          stga[:])

                for t in range(n_tiles):
                    build_rows(xT, t, T, False, t % 2)
                for t in range(w_pc):
                    build_rows(xT_local, t, Tself, True, t % 2)

    # ---- TileContext 2: attention + aggregation ----
    w_pc_run = min(w_pc, int(os.environ.get("K_WINCAP", 10**9)))
    with tile.TileContext(nc) as tc:
        with (
            tc.tile_pool(name="su2", bufs=1) as su,
            tc.tile_pool(name="ps_su2", bufs=1, space="PSUM") as psu,
            tc.tile_pool(name="sb_g", bufs=3) as sgp,
            tc.tile_pool(name="sb_w", bufs=4) as swp,
            tc.tile_pool(name="sb_sm", bufs=3) as ssp,
            tc.tile_pool(name="sb_out", bufs=2) as sop,
            tc.tile_pool(name="ps_mm", bufs=2, space="PSUM") as pmp,
        ):
            nc.gpsimd.load_library(mlp)
            iota_t = su.tile([P, P], bf16)
            nc.sync.dma_start(iota_t[:], iota_in[:])
            piota = su.tile([P, 1], f32)
            nc.sync.dma_start(piota[:], piota_in[:])
            ones1 = su.tile([1, P], f32)
            nc.vector.memset(ones1[:], 1.0)
            bias_sb = su.tile([1, D], f32)
            nc.sync.dma_start(bias_sb[:], bias_in[:])
            bias5 = su.tile([P, D], f32)
            ps_bc = psu.tile([P, D], f32)
            nc.tensor.matmul(out=ps_bc[:], lhsT=ones1[:], rhs=bias_sb[:],
                             start=True, stop=True)
            nc.vector.tensor_scalar_mul(bias5[:], ps_bc[:], float(R))

            for w in range(w_pc_run):
                rows = slice(w * P, (w + 1) * P)
                drl = ssp.tile([P, R * BT], f32, name="drl")
                nc.sync.dma_start(drl[:], drlx[rows, :])
                lo_t = ssp.tile([P, lo_cols], i16, name="lo_t")
                hi_t = ssp.tile([P, hi_cols], i16, name="hi_t")
                ad_t = ssp.tile([P, ad_cols], i16, name="ad_t")
                nc.sync.dma_start(lo_t[:], lo16[rows, :])
                nc.sync.dma_start(hi_t[:], hi16[rows, :])
                nc.sync.dma_start(ad_t[:], ad16[rows, :])
                S = ssp.tile([P, TW], bf16, name="S")
                nc.sync.dma_start(S[:], Tself[rows, :])

                # gpsimd gather ucode crashes above ~1536 idxs per call;
                # split each logical gather into rel-group slices under that.
                G_lo = sgp.tile([P, R * B1 * TW], bf16, name="G_lo")
                G_hi = sgp.tile([P, R * B2 * TW], bf16, name="G_hi")
                G_ad = sgp.tile([P, R * BT * AW], bf16, name="G_ad")
                for r0, r1 in ((0, 2), (2, 4), (4, 5)):
                    ng = r1 - r0
                    nc.gpsimd.dma_gather(
                        out_ap=G_lo[:, r0 * B1 * TW:r1 * B1 * TW].rearrange(
                            "p (j e) -> p j e", e=TW),
                        in_ap=T[0:low_cap, :],
                        idxs_ap=lo_t[:, r0 * B1 * P // 16:r1 * B1 * P // 16],
                        num_idxs=ng * B1 * P,
                        num_idxs_reg=ng * B1 * P,
                        elem_size=TW)
                    nc.gpsimd.dma_gather(
                        out_ap=G_hi[:, r0 * B2 * TW:r1 * B2 * TW].rearrange(
                            "p (j e) -> p j e", e=TW),
                        in_ap=T[h0:t_rows, :],
                        idxs_ap=hi_t[:, r0 * B2 * P // 16:r1 * B2 * P // 16],
                        num_idxs=ng * B2 * P,
                        num_idxs_reg=ng * B2 * P,
                        elem_size=TW)
                for r in range(R):
                    nc.gpsimd.dma_gather(
                        out_ap=G_ad[:, r * BT * AW:(r + 1) * BT * AW
                                    ].rearrange("p (j e) -> p j e", e=AW),
                        in_ap=Twin_a[:],
                        idxs_ap=ad_t[:, r * BT * P // 16:(r + 1) * BT * P // 16],
                        num_idxs=BT * P,
                        num_idxs_reg=BT * P,
                        elem_size=AW)

                # self-loop diagonal weights (shared across relations)
                # exp(leaky_relu(z)) == max(exp(z), exp(NEG*z)) exactly
                asum_s = ssp.tile([P, H], bf16, name="asum_s")
                nc.vector.tensor_tensor(
                    out=asum_s[:], in0=S[:, A_OFF:A_OFF + H],
                    in1=S[:, 260:262], op=mybir.AluOpType.add)
                e1_s = ssp.tile([P, H], f32, name="e1_s")
                nc.scalar.activation(e1_s[:], asum_s[:],
                                     mybir.ActivationFunctionType.Exp)
                e2_s = ssp.tile([P, H], f32, name="e2_s")
                nc.scalar.activation(e2_s[:], asum_s[:],
                                     mybir.ActivationFunctionType.Exp,
                                     scale=NEG)
                expl_s = ssp.tile([P, H], f32, name="expl_s")
                nc.vector.tensor_tensor(
                    out=expl_s[:], in0=e1_s[:], in1=e2_s[:],
                    op=mybir.AluOpType.max)
                wt_s = []
                for h in range(H):
                    w_t = swp.tile([P, P], bf16, name=f"wt_s{h}",
                                   tag=f"wt_s{h}", bufs=2)
                    nc.vector.tensor_scalar(
                        out=w_t[:], in0=iota_t[:],
                        scalar1=piota[:, 0:1],
                        scalar2=expl_s[:, h:h + 1],
                        op0=mybir.AluOpType.is_equal,
                        op1=mybir.AluOpType.mult)
                    wt_s.append(w_t)

                # edge logits for all relations at once
                G_lo3 = G_lo[:].rearrange("p (j e) -> p j e", e=TW)
                G_hi3 = G_hi[:].rearrange("p (j e) -> p j e", e=TW)
                G_ad3 = G_ad[:].rearrange("p (j e) -> p j e", e=AW)
                asum = ssp.tile([P, R * BT * H], bf16, name="asum")
                asum3 = asum[:].rearrange("p (j h) -> p j h", h=H)
                for r in range(R):
                    nc.vector.tensor_tensor(
                        out=asum3[:, r * BT:r * BT + B1, :],
                        in0=G_lo3[:, r * B1:(r + 1) * B1, A_OFF:A_OFF + H],
                        in1=G_ad3[:, r * BT:r * BT + B1, 0:H],
                        op=mybir.AluOpType.add)
                    nc.vector.tensor_tensor(
                        out=asum3[:, r * BT + B1:(r + 1) * BT, :],
                        in0=G_hi3[:, r * B2:(r + 1) * B2, A_OFF:A_OFF + H],
                        in1=G_ad3[:, r * BT + B1:(r + 1) * BT, 0:H],
                        op=mybir.AluOpType.add)
                e1 = ssp.tile([P, R * BT * H], f32, name="e1")
                nc.scalar.activation(e1[:], asum[:],
                                     mybir.ActivationFunctionType.Exp)
                e2 = ssp.tile([P, R * BT * H], f32, name="e2")
                nc.scalar.activation(e2[:], asum[:],
                                     mybir.ActivationFunctionType.Exp,
                                     scale=NEG)
                expl = ssp.tile([P, R * BT * H], f32, name="expl")
                nc.vector.tensor_tensor(
                    out=expl[:], in0=e1[:], in1=e2[:],
                    op=mybir.AluOpType.max)

                outacc = sop.tile([P, D], f32, name="outacc")
                for r in range(R):
                    ps_h = [pmp.tile([P, C + 1], f32, name=f"ps_h{h}")
                            for h in range(H)]
                    for h in range(H):
                        nc.tensor.matmul(
                            out=ps_h[h][:], lhsT=wt_s[h][:],
                            rhs=S[:, h * (C + 1):(h + 1) * (C + 1)],
                            start=True, stop=False)
                    for b in range(BT):
                        if b < B1:
                            src_t = G_lo
                            base = (r * B1 + b) * TW
                        else:
                            src_t = G_hi
                            base = (r * B2 + (b - B1)) * TW
                        for h in range(H):
                            wt = swp.tile([P, P], bf16, name="wt", tag="wt")
                            nc.vector.tensor_scalar(
                                out=wt[:], in0=iota_t[:],
                                scalar1=drl[:, r * BT + b:r * BT + b + 1],
                                scalar2=expl[:, (r * BT + b) * H + h:
                                             (r * BT + b) * H + h + 1],
                                op0=mybir.AluOpType.is_equal,
                                op1=mybir.AluOpType.mult)
                            nc.tensor.matmul(
                                out=ps_h[h][:], lhsT=wt[:],
                                rhs=src_t[:, base + h * (C + 1):
                                          base + (h + 1) * (C + 1)],
                                start=False, stop=(b == BT - 1))
                    tmp2 = ssp.tile([P, D], f32, name="tmp2", tag="tmp2",
                                    bufs=2)
                    for h in range(H):
                        recip = ssp.tile([P, 1], f32, name="recip")
                        nc.vector.reciprocal(recip[:], ps_h[h][:, C:C + 1])
                        nc.scalar.mul(tmp2[:, h * C:(h + 1) * C],
                                      ps_h[h][:, 0:C], recip[:, 0:1])
                    if r == 0:
                        nc.vector.tensor_tensor(
                            out=outacc[:], in0=tmp2[:], in1=bias5[:],
                            op=mybir.AluOpType.add)
                    else:
                        nc.vector.tensor_tensor(
                            out=outacc[:], in0=outacc[:], in1=tmp2[:],
                            op=mybir.AluOpType.add)
                nc.sync.dma_start(y[rows, :], outacc[:])

    nc.finalize()
    return nc


def _wrap16(vals):
    """[n] int array -> 16-partition-wrapped [128, n//16] int16 (replicated)."""
    n = len(vals)
    assert n % 16 == 0
    a = np.asarray(vals, np.int16).reshape(n // 16, 16).T  # [16, n//16]
    return np.tile(a, (8, 1))


def prep_inputs(inputs, ncores, low_cap=32768):
    x = np.asarray(inputs["x"], dtype=np.float32)
    N = x.shape[0]
    nw_real = -(-N // P)
    NW = -(-nw_real // ncores) * ncores
    w_pc = NW // ncores
    n_tiles = nw_real
    t_rows = n_tiles * P
    low_cap = min(low_cap, t_rows)
    h0 = t_rows - low_cap

    rels = ["parent", "child", "precede", "follow", "peer"]
    per_rel = []
    for rn in rels:
        ei = np.asarray(inputs[f"edge_index_{rn}"])
        src = ei[0].astype(np.int64)
        dst = ei[1].astype(np.int64)
        order = np.argsort(dst, kind="stable")
        src, dst = src[order], dst[order]
        w_of = dst // P
        cnt = np.bincount(w_of, minlength=NW)
        starts = np.zeros(NW + 1, np.int64)
        np.cumsum(cnt, out=starts[1:])
        per_rel.append((src, dst, starts))

    # global B1/B2 from per-(w,r) counts (real edges only; self-loops are
    # handled by the per-window diagonal block)
    must_lo_max = must_hi_max = tot_max = 0
    for src, dst, starts in per_rel:
        for w in range(NW):
            s, e = starts[w], starts[w + 1]
            sw = src[s:e]
            must_lo_max = max(must_lo_max, int((sw < h0).sum()))
            must_hi_max = max(must_hi_max, int((sw >= low_cap).sum()))
            tot_max = max(tot_max, e - s)
    B1 = max(1, -(-must_lo_max // P))
    B2 = max(1, -(-must_hi_max // P), -(-tot_max // P) - B1)
    while B1 * P < must_lo_max or (tot_max - B1 * P) > B2 * P:
        B1 += 1
    BT = B1 + B2

    lo_idx = np.zeros((NW, R, B1 * P), np.int64)
    hi_idx = np.zeros((NW, R, B2 * P), np.int64)  # pad -> hi row 0 (valid)
    ad_idx = np.zeros((NW, R, BT * P), np.int64)
    drelx = np.full((NW, R, BT * P), float(P), np.float32)
    for r, (src, dst, starts) in enumerate(per_rel):
        for w in range(NW):
            s, e = starts[w], starts[w + 1]
            sw, dw = src[s:e], dst[s:e]
            is_lo = sw < h0
            is_hi = sw >= low_cap
            flex = ~is_lo & ~is_hi
            n_lo_strict = int(is_lo.sum())
            room = B1 * P - n_lo_strict
            fi = np.flatnonzero(flex)
            lo_sel = np.concatenate([np.flatnonzero(is_lo), fi[:room]])
            hi_sel = np.concatenate([np.flatnonzero(is_hi), fi[room:]])
            assert len(lo_sel) <= B1 * P and len(hi_sel) <= B2 * P, (
                w, r, len(lo_sel), len(hi_sel))
            lo_idx[w, r, :len(lo_sel)] = sw[lo_sel]
            hi_idx[w, r, :len(hi_sel)] = sw[hi_sel] - h0
            ad_idx[w, r, :len(lo_sel)] = dw[lo_sel]
            ad_idx[w, r, B1 * P:B1 * P + len(hi_sel)] = dw[hi_sel]
            drelx[w, r, :len(lo_sel)] = (dw[lo_sel] - w * P)
            drelx[w, r, B1 * P:B1 * P + len(hi_sel)] = (dw[hi_sel] - w * P)

    nw_p = NW * P
    xT = np.zeros((D, nw_p), BF)
    xT[:, :N] = np.ascontiguousarray(x.T).astype(BF)

    shared = {
        "xT": xT,
        "Wsrc": np.ascontiguousarray(np.asarray(inputs["W_src"], np.float32)),
        "Wdst": np.ascontiguousarray(np.asarray(inputs["W_dst"], np.float32)),
        "atts": np.asarray(inputs["att_src"], np.float32).reshape(1, D).copy(),
        "attd": np.asarray(inputs["att_dst"], np.float32).reshape(1, D).copy(),
        "bias_in": np.asarray(inputs["bias"], np.float32).reshape(1, D).copy(),
        "iota_in": np.tile(np.arange(P, dtype=np.float32),
                           (P, 1)).astype(BF),
        "piota_in": np.arange(P, dtype=np.float32).reshape(P, 1).copy(),
    }

    percore = []
    for c in range(ncores):
        cb = c * w_pc * P  # first dst node owned by this core
        lo16 = np.zeros((w_pc * P, R * B1 * P // 16), np.int16)
        hi16 = np.zeros((w_pc * P, R * B2 * P // 16), np.int16)
        ad16 = np.zeros((w_pc * P, R * BT * P // 16), np.int16)
        drl = np.zeros((w_pc * P, R * BT), np.float32)
        ad_local = np.clip(ad_idx - cb, 0, w_pc * P - 1)
        for wl in range(w_pc):
            w = c * w_pc + wl
            # idx value order within a gather: (rel, block, partition)
            def fill(dst_arr, idx_src, per_rel_slots):
                vals = np.concatenate(
                    [idx_src[w, r, :per_rel_slots] for r in range(R)])
                dst_arr[wl * P:(wl + 1) * P, :] = _wrap16(vals)
            fill(lo16, lo_idx, B1 * P)
            fill(hi16, hi_idx, B2 * P)
            fill(ad16, ad_local, BT * P)
            # drel columns: [r*BT + b] value for slot block b, this partition
            drl[wl * P:(wl + 1) * P, :] = (
                drelx[w].reshape(R * BT, P).T.reshape(P, R * BT))
        percore.append({
            "lo16": lo16, "hi16": hi16, "ad16": ad16, "drlx": drl,
            "xT_local": np.ascontiguousarray(xT[:, cb:cb + w_pc * P]),
        })
    meta = dict(N=N, NW=NW, w_pc=w_pc, n_tiles=n_tiles, t_rows=t_rows,
                B1=B1, B2=B2, low_cap=low_cap, h0=h0, nw_p=nw_p)
    return meta, shared, percore


def kernel(**inputs):
    global _LAST_RESULT
    from concourse.bass_utils import run_bass_kernel_spmd

    ncores = 8
    meta, shared, percore = prep_inputs(inputs, ncores)
    key = tuple(sorted(meta.items()))
    if key not in _CACHE:
        _CACHE[key] = build_program(
            meta["n_tiles"], meta["t_rows"], meta["w_pc"], meta["B1"],
            meta["B2"], meta["low_cap"], meta["h0"], meta["nw_p"], ncores)
    nc = _CACHE[key]
    in_maps = [dict(shared, **percore[c]) for c in range(ncores)]
    res = run_bass_kernel_spmd(nc, in_maps, core_ids=list(range(ncores)),
                               **_RUN_KWARGS)
    _LAST_RESULT = res
    out = np.concatenate([res.results[c]["y"] for c in range(ncores)], axis=0)
    return np.ascontiguousarray(out[:meta["N"]])
